# revision 61
# baseline (speedup 1.0000x reference)
"""FAVOR+ (Performer) attention kernel for 8 Trainium2 NeuronCores.

Problem: B=4, N=4096, D=512, H=8, DK=64, M=128 (nb_features=256), fp32 in/out.

Sharding: 8 cores = 4 batches x 2 head-groups (4 heads each).  Each core
computes, for its (batch, 4-head) shard, the full FAVOR pipeline and
writes a feature-major partial output yT (512, 4096) in bf16; the host
sums the two head-group partials per batch and transposes back.

v2 design (bf16 compute):
  * all matmuls in bf16 (fp32 PSUM accumulate): 1 cyc/row on the PE and
    FWL-accelerated weight loads, vs 2 cyc/row + double loads for fp32r
  * the reference's +EPS on the denominator is dropped: with the q-side
    prefactor cancelled, den is O(300..1e5) while the exact correction
    E = eps*sqrt(NB)*exp(shift_q+ssq_q/2) is O(100) -> 5e-3 rel effect
    (measured), far inside the 2e-2 gate
  * k-side per-token bias exp(-(shift+ssq+ln sqrt(NB))) is NOT folded
    into the phi exps (which would force 8 small biased activations per
    chunk); instead phi_k = exp(+-c*proj) unbiased as two [128,512]
    activations, and the bias factor e^{bk} scales the v-columns (and
    becomes the ksum column), which is algebraically identical
  * kv is accumulated directly transposed (lhsT=phi chunk, rhs=v_aug) so
    no PE-transpose pass is needed; kv[128m, 2j, 64d+1] with ksum col 64
  * phase B: den = row 64 of the num PSUM tile; 1/den via DVE reciprocal
    straight out of PSUM, broadcast across partitions with a K=1
    ones-matmul, applied in the PSUM->SBUF copy of num (fused)
"""

import contextlib
import sys

if "/opt/trn_rl_repo" not in sys.path:
    sys.path.insert(0, "/opt/trn_rl_repo")

import numpy as np
import ml_dtypes

import concourse.bass as bass
import concourse.tile as tile
from concourse import mybir

B, N, D = 4, 4096, 512
H, DK = 8, 64
M = 128
NB = 2 * M
F32 = mybir.dt.float32
BF16 = mybir.dt.bfloat16

INV_DKRT = float(1.0 / (DK ** 0.25))
LN_SQRT_NB = float(np.log(np.sqrt(NB)))      # ln 16
SSQ_C = float(1.0 / (2.0 * np.sqrt(DK)))     # sum k^2 -> 0.5*||k/dk^.25||^2

TOK_CH = N // 128   # 32 token chunks of 128
TOK_B = N // 512    # 8 token blocks of 512

AF = mybir.ActivationFunctionType
ALU = mybir.AluOpType
AX = mybir.AxisListType


def _split_waits(nc, maxw=1):
    """walrus in this container allows a single embedded sem wait per
    instruction; the Tile exit drain carries several.  Hoist extras onto
    preceding NoOps on the same engine."""
    for _bbname, bb in nc.bb_map.items():
        insts = bb.bb.instructions
        out = []
        for inst in insts:
            si = inst.sync_info
            if si and si.on_wait and len(si.on_wait) > maxw:
                waits = list(si.on_wait)
                k = 0
                while len(waits) > maxw:
                    chunk, waits = waits[:maxw], waits[maxw:]
                    nop = mybir.InstNoOp(
                        name=f"{inst.name}-wsplit{k}", ins=[], outs=[]
                    )
                    k += 1
                    nop.engine = inst.engine
                    nop.sync_info = mybir.SyncInfo(on_wait=chunk, on_update=[])
                    out.append(nop)
                inst.sync_info = mybir.SyncInfo(
                    on_wait=waits, on_update=list(si.on_update or [])
                )
            out.append(inst)
        insts[:] = out


def build_program(use_bv=False, use_mask=False, use_bqk=False, split=True, debug=False):
    nc = bass.Bass()
    if debug:
        dbg_qk = nc.declare_dram_parameter("dbg_qk", (4, 128, N), BF16, isOutput=True)
        dbg_ch = nc.declare_dram_parameter("dbg_ch", (6, 128, 512), F32, isOutput=True)
        dbg_kvs = nc.declare_dram_parameter("dbg_kvs", (4, 128, 130), BF16, isOutput=True)
        dbg_b = nc.declare_dram_parameter("dbg_b", (8, 128, 1024), BF16, isOutput=True)

    xT = nc.declare_dram_parameter("xT", (D, N), BF16, isOutput=False)
    wqk_d = nc.declare_dram_parameter("wqk", (D, 512), BF16, isOutput=False)
    wv_d = nc.declare_dram_parameter("wv", (D, 256), BF16, isOutput=False)
    womq_d = nc.declare_dram_parameter("womq", (128, 512), BF16, isOutput=False)
    womk_d = nc.declare_dram_parameter("womk", (128, 512), BF16, isOutput=False)
    wy_d = nc.declare_dram_parameter("wy", (256, 512), BF16, isOutput=False)
    bqk_d = nc.declare_dram_parameter("bqk", (128, 4), F32, isOutput=False)
    # consts cols: [0:2] ssq_blk (SSQ_C at partitions 0:64 / 64:128),
    # [4:132] all-ones on partitions 0 and 64 (bv lhsT)
    consts_d = nc.declare_dram_parameter("consts", (128, 132), BF16, isOutput=False)
    if use_bv:
        bv_d = nc.declare_dram_parameter("bv", (1, 256), BF16, isOutput=False)
    if use_mask:
        valid_d = nc.declare_dram_parameter("valid", (128, TOK_CH), F32, isOutput=False)
    yT = nc.declare_dram_parameter("yT", (D, N), BF16, isOutput=True)

    with tile.TileContext(nc) as tc, contextlib.ExitStack() as ctx:
        wpool = ctx.enter_context(tc.tile_pool(name="weights", bufs=1))
        qkpool = ctx.enter_context(tc.tile_pool(name="qk", bufs=1))
        kvsp = ctx.enter_context(tc.tile_pool(name="kvs", bufs=1))

        # ---- weights / consts ---------------------------------------
        t_wqk = [wpool.tile([128, 512], BF16, tag=f"wqk{k}", name=f"wqk{k}") for k in range(4)]
        t_wv = [wpool.tile([128, 256], BF16, tag=f"wv{k}", name=f"wv{k}") for k in range(4)]
        for k in range(4):
            nc.sync.dma_start(out=t_wqk[k], in_=wqk_d[128 * k:128 * (k + 1), :])
            nc.sync.dma_start(out=t_wv[k], in_=wv_d[128 * k:128 * (k + 1), :])
        t_womq = wpool.tile([128, 512], BF16, tag="womq", name="womq")
        nc.sync.dma_start(out=t_womq, in_=womq_d[:, :])
        t_womk = wpool.tile([128, 512], BF16, tag="womk", name="womk")
        nc.sync.dma_start(out=t_womk, in_=womk_d[:, :])
        t_wy = [wpool.tile([128, 512], BF16, tag=f"wy{k}", name=f"wy{k}") for k in range(2)]
        for k in range(2):
            nc.sync.dma_start(out=t_wy[k], in_=wy_d[128 * k:128 * (k + 1), :])
        t_bqk = wpool.tile([128, 4], F32, tag="bqk", name="bqk")
        nc.sync.dma_start(out=t_bqk, in_=bqk_d[:, :])
        t_consts = wpool.tile([128, 132], BF16, tag="consts", name="consts")
        nc.sync.dma_start(out=t_consts, in_=consts_d[:, :])
        ones_blk = t_consts[:, 0:2]          # [128,2] (SSQ_C-valued)
        t_lnc = wpool.tile([128, 1], F32, tag="lnc", name="lnc")
        nc.vector.memset(t_lnc, -LN_SQRT_NB)
        if use_bv:
            t_bv = wpool.tile([1, 256], BF16, tag="bv", name="bv")
            nc.sync.dma_start(out=t_bv, in_=bv_d[:, :])
            ones_row0 = t_consts[0:1, 4:132]  # [1,128] ones at partition 0
        if use_mask:
            t_valid = wpool.tile([128, TOK_CH], F32, tag="valid", name="valid")
            nc.sync.dma_start(out=t_valid, in_=valid_d[:, :])

        # qk[m]: feature-major q/k, bf16; m=0,1 -> q heads (0,1),(2,3);
        # m=2,3 -> k heads (0,1),(2,3)
        t_qk = [qkpool.tile([128, N], BF16, tag=f"qk{m}", name=f"qk{m}") for m in range(4)]
        # kv (transposed) + ksum col 64, bf16, per head -- filled in phase A
        t_kvTs = [kvsp.tile([128, 2, 65], BF16, tag=f"kvTs{h}", name=f"kvTs{h}") for h in range(4)]
        # q-side features for t8 blocks 0..3, precomputed during phase A
        # (they do not depend on kv) to offload phase B's ACT bottleneck
        QPRE = 4
        t_qpre = [[kvsp.tile([128, 2, 512], BF16, tag=f"qpre{t8}_{h}",
                             name=f"qpre{t8}_{h}") for h in range(4)]
                  for t8 in range(QPRE)]

        # ---- phase A: S1a projections + k-side features + kv --------
        with tc.tile_pool(name="xt", bufs=1) as xtp, \
             tc.tile_pool(name="worka", bufs=2) as wka, \
             tc.tile_pool(name="psA", bufs=2, space="PSUM") as psA:

            # xt in column-quarters, dispatched from the (idle) ACT queue:
            # the first S1a-k blocks only need the first 1024 tokens, and
            # the SP queue stays free for the weight DMAs
            t_xt = [[xtp.tile([128, N // 4], BF16, tag=f"xt{k}_{hf}", name=f"xt{k}_{hf}")
                     for hf in range(4)] for k in range(4)]
            for hf in range(4):
                for k in range(4):
                    # first quarter on the SP queue (fires immediately),
                    # the rest from the ACT queue in parallel
                    eng = nc.sync if hf == 0 else nc.scalar
                    eng.dma_start(
                        out=t_xt[k][hf],
                        in_=xT[128 * k:128 * (k + 1),
                               1024 * hf:1024 * (hf + 1)])

            def xtap(k, sl):
                hf, lo = divmod(sl.start, 1024)
                return t_xt[k][hf][:, lo:lo + (sl.stop - sl.start)]

            # kv accumulators, two heads packed per bank:
            # [128 m(j-half), 2 head-in-pair, 2 j, 64 d + ksum]
            t_kv2 = [psA.tile([128, 2, 2, 65], F32, tag=f"kv{g}", name=f"kv{g}", bufs=1)
                     for g in range(2)]

            def s1a_block(m, t8, on_dve=False):
                sl = slice(512 * t8, 512 * (t8 + 1))
                ps = psA.tile([128, 512], F32, tag="s1a", name=f"s1a_{m}_{t8}")
                for k in range(4):
                    nc.tensor.matmul(
                        ps,
                        lhsT=t_wqk[k][:, 128 * m:128 * (m + 1)],
                        rhs=xtap(k, sl),
                        start=(k == 0),
                        stop=(k == 3),
                    )
                if on_dve and not use_bqk:
                    nc.vector.tensor_copy(out=t_qk[m][:, sl], in_=ps)
                else:
                    nc.scalar.activation(
                        out=t_qk[m][:, sl], in_=ps,
                        func=AF.Identity,
                        bias=t_bqk[:, m:m + 1], scale=1.0,
                    )

            def qpre_block(t8, h):
                sl = slice(512 * t8, 512 * (t8 + 1))
                pq = psA.tile([128, 512], F32, tag="s1a", name=f"pqp{t8}_{h}")
                nc.tensor.matmul(
                    pq,
                    lhsT=t_womq[:, 128 * h:128 * (h + 1)],
                    rhs=t_qk[h // 2][:, sl],
                    start=True, stop=True,
                )
                nc.scalar.activation(
                    out=t_qpre[t8][h][:, 0, :], in_=pq,
                    func=AF.Exp, bias=0.0, scale=INV_DKRT,
                )
                nc.scalar.activation(
                    out=t_qpre[t8][h][:, 1, :], in_=pq,
                    func=AF.Exp, bias=0.0, scale=-INV_DKRT,
                )

            # k-feature blocks first, quarter-major so compute starts as
            # soon as the first xt quarter lands; q blocks + qp-precompute
            # are interleaved into the chunk loop below as PE filler
            for q in range(4):
                for m in (2, 3):
                    for t8 in (2 * q, 2 * q + 1):
                        s1a_block(m, t8)

            s1aq = [(m, t8) for m in (0, 1) for t8 in range(TOK_B)]
            # (t8, h) qp-precompute units, ordered so the needed qk-q
            # block (m=h//2, t8) is already emitted by its chunk slot
            qpre_units = [(t8, h) for h in (0, 1) for t8 in range(QPRE)] + \
                         [(t8, h) for h in (2, 3) for t8 in range(QPRE)]
            for t in range(TOK_CH):
                cl = slice(128 * t, 128 * (t + 1))
                # v chunk token-major (cols 0:256); sum k^2 in 256:260
                pv = psA.tile([128, 260], F32, tag="pv", name=f"pv{t}")
                for k in range(4):
                    nc.tensor.matmul(
                        pv[:, 0:256],
                        lhsT=xtap(k, cl), rhs=t_wv[k],
                        start=(k == 0), stop=(k == 3) and not use_bv,
                    )
                if use_bv:
                    nc.tensor.matmul(
                        pv[:, 0:256],
                        lhsT=ones_row0, rhs=t_bv,
                        start=False, stop=True,
                    )
                # proj_k token-major via blockdiag omega: [128 tok, 4h*128m]
                pk = psA.tile([128, 512], F32, tag="pk", name=f"pk{t}")
                for p in range(2):
                    nc.tensor.matmul(
                        pk[:, 256 * p:256 * (p + 1)],
                        lhsT=t_qk[2 + p][:, cl],
                        rhs=t_womk[:, 256 * p:256 * (p + 1)],
                        start=True, stop=True,
                    )
                # sum k^2 per token/head via ones-matmul on squared kT chunk
                # (squares on the otherwise-idle GPSIMD engine)
                ksq = wka.tile([128, 2, 128], BF16, tag="ksq", name=f"ksq{t}")
                for p in range(2):
                    nc.gpsimd.tensor_tensor(
                        out=ksq[:, p, :],
                        in0=t_qk[2 + p][:, cl], in1=t_qk[2 + p][:, cl],
                        op=ALU.mult,
                    )
                for p in range(2):
                    nc.tensor.matmul(
                        pv[:, 256 + 2 * p:258 + 2 * p],
                        lhsT=ksq[:, p, :], rhs=ones_blk,
                        start=True, stop=True, skip_group_check=True,
                    )
                # shift_k = absmax_m proj (per token, head)
                srd = wka.tile([128, 4], F32, tag="srd", name=f"srd{t}")
                nc.vector.tensor_reduce(
                    out=srd,
                    in_=pk.rearrange("p (h m) -> p h m", h=4),
                    axis=AX.X, op=ALU.max,
                    apply_absolute_value=True,
                )
                # bk0 = -c*srd - SSQ_C*sumk2 (the ssq_blk "ones" carry
                # SSQ_C); ebk = exp(bk0 - ln sqrt(NB)) via the ACT bias
                bk = wka.tile([128, 4], F32, tag="bk", name=f"bk{t}")
                nc.vector.scalar_tensor_tensor(
                    out=bk, in0=srd, scalar=-INV_DKRT, in1=pv[:, 256:260],
                    op0=ALU.mult, op1=ALU.subtract,
                )
                ebk = wka.tile([128, 4, 1], BF16, tag="ebk", name=f"ebk{t}")
                nc.scalar.activation(
                    out=ebk, in_=bk, func=AF.Exp,
                    bias=t_lnc, scale=1.0,
                )
                if use_mask:
                    nc.vector.tensor_scalar_mul(
                        ebk.rearrange("p h o -> p (h o)"),
                        ebk.rearrange("p h o -> p (h o)"),
                        t_valid[:, t:t + 1])
                # scale v rows by ebk per head: one DVE broadcast-multiply
                # straight from PSUM
                va = wka.tile([128, 4, 65], BF16, tag="va", name=f"va{t}")
                nc.vector.tensor_tensor(
                    out=va[:, :, 0:64],
                    in0=pv[:, 0:256].rearrange("p (h d) -> p h d", h=4),
                    in1=ebk.to_broadcast((128, 4, 64)),
                    op=ALU.mult,
                )
                nc.gpsimd.tensor_copy(
                    out=va[:, :, 64:65].rearrange("p h o -> p (h o)"),
                    in_=ebk.rearrange("p h o -> p (h o)"))
                # phi_k = exp(+-c*proj), unbiased, bf16
                kph = wka.tile([128, 4, 256], BF16, tag="kph", name=f"kph{t}")
                nc.scalar.activation(
                    out=kph[:, :, 0:128],
                    in_=pk.rearrange("p (h m) -> p h m", h=4),
                    func=AF.Exp, bias=0.0, scale=INV_DKRT,
                )
                nc.scalar.activation(
                    out=kph[:, :, 128:256],
                    in_=pk.rearrange("p (h m) -> p h m", h=4),
                    func=AF.Exp, bias=0.0, scale=-INV_DKRT,
                )
                # kv accumulation, directly transposed:
                # kvT[m, (d|ksum)] += phi[tok, m].T @ va[tok, (d|ksum)]
                # start only on the bank's first group: its start marks the
                # whole 2KB bank pending-zero, initializing all 4 groups --
                # a second start would re-mark (and discard) earlier writes
                for h in range(4):
                    for j in range(2):
                        nc.tensor.matmul(
                            t_kv2[h // 2][:, h % 2, j, :],
                            lhsT=kph[:, h, 128 * j:128 * (j + 1)],
                            rhs=va[:, h, :],
                            start=(t == 0 and h % 2 == 0 and j == 0),
                            stop=(t == TOK_CH - 1),
                            skip_group_check=True,
                        )
                if debug and t == 0:
                    dpk = wka.tile([128, 512], F32, tag="dpk", name="dpk")
                    nc.vector.tensor_copy(out=dpk, in_=pk)
                    nc.sync.dma_start(out=dbg_ch[0], in_=dpk)
                    dpv = wka.tile([128, 512], F32, tag="dpk", name="dpv")
                    nc.vector.tensor_copy(out=dpv[:, 0:260], in_=pv)
                    nc.sync.dma_start(out=dbg_ch[1][:, 0:260], in_=dpv[:, 0:260])
                    dsm = wka.tile([128, 512], F32, tag="dpk", name="dsm")
                    nc.vector.tensor_copy(out=dsm[:, 0:4], in_=srd)
                    nc.vector.tensor_copy(out=dsm[:, 8:12], in_=bk)
                    nc.vector.tensor_copy(
                        out=dsm[:, 12:16],
                        in_=ebk.rearrange("p h o -> p (h o)"))
                    nc.sync.dma_start(out=dbg_ch[2][:, 0:16], in_=dsm[:, 0:16])
                    dkb = wka.tile([128, 1024], BF16, tag="dkb", name="dkb")
                    nc.vector.tensor_copy(
                        out=dkb[:, 256:516],
                        in_=va.rearrange("p a b -> p (a b)"))
                    nc.sync.dma_start(out=dbg_b[6][:, 256:516], in_=dkb[:, 256:516])
                    dk2 = wka.tile([128, 1024], BF16, tag="dkb", name="dk2")
                    nc.vector.tensor_copy(
                        out=dk2, in_=kph.rearrange("p a b -> p (a b)"))
                    nc.sync.dma_start(out=dbg_b[7], in_=dk2)
                # PE filler: S1a q-feature blocks at odd chunks, qp
                # precompute at even chunks once the q features exist
                if t % 2 == 1 and s1aq:
                    s1a_block(*s1aq.pop(0), on_dve=True)
                elif t % 2 == 0 and t >= 4 and qpre_units:
                    qpre_block(*qpre_units.pop(0))

            while s1aq:
                s1a_block(*s1aq.pop(0), on_dve=True)
            while qpre_units:
                qpre_block(*qpre_units.pop(0))

            for h in range(4):
                nc.vector.tensor_copy(out=t_kvTs[h], in_=t_kv2[h // 2][:, h % 2, :, :])
            if debug:
                for m in range(4):
                    nc.sync.dma_start(out=dbg_qk[m], in_=t_qk[m])
                for h in range(4):
                    nc.sync.dma_start(
                        out=dbg_kvs[h],
                        in_=t_kvTs[h].rearrange("p a b -> p (a b)"))

        # ---- phase B: q-side features, num/den, output --------------
        with tc.tile_pool(name="workb", bufs=2) as wkb, \
             tc.tile_pool(name="drb", bufs=2, space="DRAM") as drb, \
             tc.tile_pool(name="psB", bufs=2, space="PSUM") as psB:
            # den4/rcp4 ring slots: unused partition rows must hold a
            # finite value (the Ln/Exp pass covers all 128 rows)
            den4s = [wkb.tile([128, 512], F32, tag="den4", name=f"den4_{i}")
                     for i in range(2)]
            for i in range(2):
                nc.vector.memset(den4s[i], 1.0)
            for t8 in range(TOK_B):
                sl = slice(512 * t8, 512 * (t8 + 1))
                pns = []
                den4 = den4s[t8 % 2]
                drcp = drb.tile([4, 512], BF16, tag="drcp", name=f"drcp{t8}")
                for h in range(4):
                    if t8 < QPRE:
                        qp = t_qpre[t8][h]
                    else:
                        pq = psB.tile([128, 512], F32, tag="pq", name=f"pq{t8}_{h}", bufs=3)
                        nc.tensor.matmul(
                            pq,
                            lhsT=t_womq[:, 128 * h:128 * (h + 1)],
                            rhs=t_qk[h // 2][:, sl],
                            start=True, stop=True,
                        )
                        qp = wkb.tile([128, 2, 512], BF16, tag="qp", name=f"qp{t8}_{h}")
                        nc.scalar.activation(
                            out=qp[:, 0, :], in_=pq,
                            func=AF.Exp, bias=0.0, scale=INV_DKRT,
                        )
                        nc.scalar.activation(
                            out=qp[:, 1, :], in_=pq,
                            func=AF.Exp, bias=0.0, scale=-INV_DKRT,
                        )
                    pn = psB.tile([65, 512], F32, tag="pn", name=f"pn{t8}_{h}", bufs=3)
                    for j in range(2):
                        nc.tensor.matmul(
                            pn,
                            lhsT=t_kvTs[h][:, j, :], rhs=qp[:, j, :],
                            start=(j == 0), stop=(j == 1),
                        )
                    pns.append(pn)
                    # gather den rows at 32-aligned partitions; 1/den is
                    # computed as exp(-ln(den)) on ACT (ln+exp share one
                    # activation table with exp/identity -- no reloads;
                    # DVE reciprocal measures 3.3us per call)
                    nc.vector.tensor_copy(
                        out=den4[32 * h:32 * h + 1, :], in_=pn[64:65, :])
                    if debug and t8 == 0:
                        nc.sync.dma_start(
                            out=dbg_b[h],
                            in_=qp.rearrange("p a b -> p (a b)"))
                lnd = wkb.tile([128, 512], F32, tag="lnd", name=f"lnd{t8}")
                nc.scalar.activation(
                    out=lnd, in_=den4, func=AF.Ln, bias=0.0, scale=1.0,
                )
                rcp4 = wkb.tile([128, 512], BF16, tag="rcp4", name=f"rcp4{t8}")
                nc.scalar.activation(
                    out=rcp4, in_=lnd, func=AF.Exp, bias=0.0, scale=-1.0,
                )
                # dispatch from ACT: no cross-engine hop after the exp
                nc.scalar.dma_start(
                    out=drcp,
                    in_=rcp4.rearrange("(a b) f -> a b f", b=32)[:, 0, :])
                ns = []
                for d in range(2):
                    # copy num out of PSUM immediately (frees the pn ring
                    # for the next block); the 1/den multiply runs on the
                    # otherwise-idle GPSIMD from SBUF
                    nsr = wkb.tile([128, 512], BF16, tag="nsr", name=f"nsr{t8}_{d}", bufs=4)
                    nc.vector.tensor_copy(
                        out=nsr[0:64, :], in_=pns[2 * d][0:64, :])
                    nc.vector.tensor_copy(
                        out=nsr[64:128, :], in_=pns[2 * d + 1][0:64, :])
                    pbs = wkb.tile([128, 512], BF16, tag="pbs", name=f"pbs{t8}_{d}", bufs=4)
                    nc.sync.dma_start(
                        out=pbs[0:64, :],
                        in_=drcp[2 * d:2 * d + 1, :].to_broadcast((64, 512)))
                    nc.sync.dma_start(
                        out=pbs[64:128, :],
                        in_=drcp[2 * d + 1:2 * d + 2, :].to_broadcast((64, 512)))
                    # all-bf16 SBUF operands -> DVE 4x mode (~190ns)
                    nst = wkb.tile([128, 512], BF16, tag="ns", name=f"ns{t8}_{d}", bufs=4)
                    nc.vector.tensor_tensor(
                        out=nst, in0=nsr, in1=pbs, op=ALU.mult,
                    )
                    ns.append(nst)
                for m4 in range(4):
                    py = psB.tile([128, 512], F32, tag="py", name=f"py{t8}_{m4}")
                    for dd in range(2):
                        nc.tensor.matmul(
                            py,
                            lhsT=t_wy[dd][:, 128 * m4:128 * (m4 + 1)],
                            rhs=ns[dd],
                            start=(dd == 0), stop=(dd == 1),
                        )
                    ysb = wkb.tile([128, 512], BF16, tag="ysb", name=f"ysb{t8}_{m4}", bufs=4)
                    nc.vector.tensor_copy(out=ysb, in_=py)
                    nc.sync.dma_start(
                        out=yT[128 * m4:128 * (m4 + 1), sl], in_=ysb,
                    )
                if debug and t8 == 0:
                    nc.sync.dma_start(out=dbg_b[4][:, 0:512], in_=ns[0])
                    nc.sync.dma_start(out=dbg_b[5][:, 0:512], in_=ns[1])

    if split:
        _split_waits(nc)
    return nc


_PROGRAM_CACHE = {}


def _get_program(use_bv, use_mask, use_bqk):
    key = (use_bv, use_mask, use_bqk)
    if key not in _PROGRAM_CACHE:
        _PROGRAM_CACHE[key] = build_program(*key)
    return _PROGRAM_CACHE[key]


def _bf(a):
    return np.ascontiguousarray(a).astype(ml_dtypes.bfloat16)


def make_in_maps(x, key_padding_mask, Wqkv, bqkv, Wout, bout, omega):
    """Shard + lay out the full inputs into 8 per-core input maps."""
    Wq, Wk, Wv = Wqkv[0:D], Wqkv[D:2 * D], Wqkv[2 * D:3 * D]
    bq, bk_, bv = bqkv[0:D], bqkv[D:2 * D], bqkv[2 * D:3 * D]
    mask = key_padding_mask

    use_bv = bool(np.any(bv != 0))
    use_mask = bool(np.any(mask))
    use_bqk = bool(np.any(bq != 0) or np.any(bk_ != 0))

    consts = np.zeros((128, 132), np.float32)
    consts[0:64, 0] = SSQ_C
    consts[64:128, 1] = SSQ_C
    consts[0, 4:132] = 1.0
    consts[64, 4:132] = 1.0

    in_maps = []
    for c in range(8):
        b, hg = c // 2, c % 2
        dsl = slice(256 * hg, 256 * (hg + 1))
        heads = [4 * hg + i for i in range(4)]
        wqk_c = np.concatenate([Wq.T[:, dsl], Wk.T[:, dsl]], axis=1)
        womq_c = np.zeros((128, 512), np.float32)
        womk_c = np.zeros((128, 512), np.float32)
        for i, g in enumerate(heads):
            off = 64 * (i % 2)
            womq_c[off:off + 64, 128 * i:128 * (i + 1)] = omega[g].T
        for p in range(2):
            womk_c[0:64, 256 * p:256 * p + 128] = omega[heads[2 * p]].T
            womk_c[64:128, 256 * p + 128:256 * p + 256] = omega[heads[2 * p + 1]].T
        bqk_vec = np.concatenate([bq[dsl], bk_[dsl]])
        im = {
            "xT": _bf(x[b].T),
            "wqk": _bf(wqk_c),
            "wv": _bf(Wv.T[:, dsl]),
            "womq": _bf(womq_c),
            "womk": _bf(womk_c),
            "wy": _bf(Wout[:, dsl].T),
            "bqk": np.ascontiguousarray(bqk_vec.reshape(4, 128).T),
            "consts": _bf(consts),
        }
        if use_bv:
            im["bv"] = _bf(bv[None, :])
        if use_mask:
            im["valid"] = np.ascontiguousarray(
                (~mask[b]).astype(np.float32).reshape(TOK_CH, 128).T
            )
        in_maps.append(im)
    return in_maps, (use_bv, use_mask, use_bqk)


def gather_output(per_core_yT, bout):
    """Sum head-group partials, transpose back to (B, N, D), add bout."""
    y = np.empty((B, N, D), np.float32)
    for b in range(B):
        acc = (per_core_yT[2 * b].astype(np.float32)
               + per_core_yT[2 * b + 1].astype(np.float32))
        y[b] = acc.T
    if np.any(bout != 0):
        y += bout[None, None, :]
    return y


def kernel(x, key_padding_mask, Wqkv, bqkv, Wout, bout, omega):
    from concourse.bass_utils import run_bass_kernel_spmd

    x = np.asarray(x, np.float32)
    mask = np.asarray(key_padding_mask)
    Wqkv = np.asarray(Wqkv, np.float32)
    bqkv = np.asarray(bqkv, np.float32)
    Wout = np.asarray(Wout, np.float32)
    bout = np.asarray(bout, np.float32)
    omega = np.asarray(omega, np.float32)

    in_maps, flags = make_in_maps(x, mask, Wqkv, bqkv, Wout, bout, omega)
    nc = _get_program(*flags)
    res = run_bass_kernel_spmd(nc, in_maps, list(range(8)))
    return gather_output([r["yT"] for r in res.results], bout)


# revision 72
# speedup vs baseline: 1.0992x; 1.0992x over previous
"""FAVOR+ (Performer) attention kernel for 8 Trainium2 NeuronCores.

Problem: B=4, N=4096, D=512, H=8, DK=64, M=128 (nb_features=256), fp32 in/out.

Sharding: 8 cores = 4 batches x 2 head-groups (4 heads each).  Each core
computes, for its (batch, 4-head) shard, the full FAVOR pipeline and
writes a feature-major partial output yT (512, 4096) in bf16; the host
sums the two head-group partials per batch and transposes back.

v2 design (bf16 compute):
  * all matmuls in bf16 (fp32 PSUM accumulate): 1 cyc/row on the PE and
    FWL-accelerated weight loads, vs 2 cyc/row + double loads for fp32r
  * the reference's +EPS on the denominator is dropped: with the q-side
    prefactor cancelled, den is O(300..1e5) while the exact correction
    E = eps*sqrt(NB)*exp(shift_q+ssq_q/2) is O(100) -> 5e-3 rel effect
    (measured), far inside the 2e-2 gate
  * k-side per-token bias exp(-(shift+ssq+ln sqrt(NB))) is NOT folded
    into the phi exps (which would force 8 small biased activations per
    chunk); instead phi_k = exp(+-c*proj) unbiased as two [128,512]
    activations, and the bias factor e^{bk} scales the v-columns (and
    becomes the ksum column), which is algebraically identical
  * kv is accumulated directly transposed (lhsT=phi chunk, rhs=v_aug) so
    no PE-transpose pass is needed; kv[128m, 2j, 64d+1] with ksum col 64
  * phase B: den = row 64 of the num PSUM tile; 1/den via DVE reciprocal
    straight out of PSUM, broadcast across partitions with a K=1
    ones-matmul, applied in the PSUM->SBUF copy of num (fused)
"""

import contextlib
import sys

if "/opt/trn_rl_repo" not in sys.path:
    sys.path.insert(0, "/opt/trn_rl_repo")

import numpy as np
import ml_dtypes

import concourse.bass as bass
import concourse.tile as tile
from concourse import mybir

B, N, D = 4, 4096, 512
H, DK = 8, 64
M = 128
NB = 2 * M
F32 = mybir.dt.float32
BF16 = mybir.dt.bfloat16

INV_DKRT = float(1.0 / (DK ** 0.25))
LN_SQRT_NB = float(np.log(np.sqrt(NB)))      # ln 16
SSQ_C = float(1.0 / (2.0 * np.sqrt(DK)))     # sum k^2 -> 0.5*||k/dk^.25||^2

TOK_CH = N // 128   # 32 token chunks of 128
TOK_B = N // 512    # 8 token blocks of 512

AF = mybir.ActivationFunctionType
ALU = mybir.AluOpType
AX = mybir.AxisListType


def _split_waits(nc, maxw=1):
    """walrus in this container allows a single embedded sem wait per
    instruction; the Tile exit drain carries several.  Hoist extras onto
    preceding NoOps on the same engine."""
    for _bbname, bb in nc.bb_map.items():
        insts = bb.bb.instructions
        out = []
        for inst in insts:
            si = inst.sync_info
            if si and si.on_wait and len(si.on_wait) > maxw:
                waits = list(si.on_wait)
                k = 0
                while len(waits) > maxw:
                    chunk, waits = waits[:maxw], waits[maxw:]
                    nop = mybir.InstNoOp(
                        name=f"{inst.name}-wsplit{k}", ins=[], outs=[]
                    )
                    k += 1
                    nop.engine = inst.engine
                    nop.sync_info = mybir.SyncInfo(on_wait=chunk, on_update=[])
                    out.append(nop)
                inst.sync_info = mybir.SyncInfo(
                    on_wait=waits, on_update=list(si.on_update or [])
                )
            out.append(inst)
        insts[:] = out


def build_program(use_bv=False, use_mask=False, use_bqk=False, split=True, debug=False):
    nc = bass.Bass()
    if debug:
        dbg_qk = nc.declare_dram_parameter("dbg_qk", (4, 128, N), BF16, isOutput=True)
        dbg_ch = nc.declare_dram_parameter("dbg_ch", (6, 128, 512), F32, isOutput=True)
        dbg_kvs = nc.declare_dram_parameter("dbg_kvs", (4, 128, 130), BF16, isOutput=True)
        dbg_b = nc.declare_dram_parameter("dbg_b", (8, 128, 1024), BF16, isOutput=True)

    xT = nc.declare_dram_parameter("xT", (D, N), BF16, isOutput=False)
    wqk_d = nc.declare_dram_parameter("wqk", (D, 512), BF16, isOutput=False)
    wv_d = nc.declare_dram_parameter("wv", (D, 256), BF16, isOutput=False)
    womq_d = nc.declare_dram_parameter("womq", (128, 512), BF16, isOutput=False)
    womk_d = nc.declare_dram_parameter("womk", (128, 512), BF16, isOutput=False)
    wy_d = nc.declare_dram_parameter("wy", (256, 512), BF16, isOutput=False)
    bqk_d = nc.declare_dram_parameter("bqk", (128, 4), F32, isOutput=False)
    # consts cols: [0:2] ssq_blk (SSQ_C at partitions 0:64 / 64:128),
    # [4:132] all-ones on partitions 0 and 64 (bv lhsT)
    consts_d = nc.declare_dram_parameter("consts", (128, 132), BF16, isOutput=False)
    if use_bv:
        bv_d = nc.declare_dram_parameter("bv", (1, 256), BF16, isOutput=False)
    if use_mask:
        valid_d = nc.declare_dram_parameter("valid", (128, TOK_CH), F32, isOutput=False)
    yT = nc.declare_dram_parameter("yT", (D, N), BF16, isOutput=True)

    with tile.TileContext(nc) as tc, contextlib.ExitStack() as ctx:
        wpool = ctx.enter_context(tc.tile_pool(name="weights", bufs=1))
        qkpool = ctx.enter_context(tc.tile_pool(name="qk", bufs=1))
        kvsp = ctx.enter_context(tc.tile_pool(name="kvs", bufs=1))

        # ---- weights / consts ---------------------------------------
        t_wqk = [wpool.tile([128, 512], BF16, tag=f"wqk{k}", name=f"wqk{k}") for k in range(4)]
        t_wv = [wpool.tile([128, 256], BF16, tag=f"wv{k}", name=f"wv{k}") for k in range(4)]
        for k in range(4):
            nc.sync.dma_start(out=t_wqk[k], in_=wqk_d[128 * k:128 * (k + 1), :])
            nc.sync.dma_start(out=t_wv[k], in_=wv_d[128 * k:128 * (k + 1), :])
        t_womq = wpool.tile([128, 512], BF16, tag="womq", name="womq")
        nc.sync.dma_start(out=t_womq, in_=womq_d[:, :])
        t_womk = wpool.tile([128, 512], BF16, tag="womk", name="womk")
        nc.sync.dma_start(out=t_womk, in_=womk_d[:, :])
        t_wy = [wpool.tile([128, 512], BF16, tag=f"wy{k}", name=f"wy{k}") for k in range(2)]
        for k in range(2):
            nc.sync.dma_start(out=t_wy[k], in_=wy_d[128 * k:128 * (k + 1), :])
        t_bqk = wpool.tile([128, 4], F32, tag="bqk", name="bqk")
        nc.sync.dma_start(out=t_bqk, in_=bqk_d[:, :])
        t_consts = wpool.tile([128, 132], BF16, tag="consts", name="consts")
        nc.sync.dma_start(out=t_consts, in_=consts_d[:, :])
        ones_blk = t_consts[:, 0:2]          # [128,2] (SSQ_C-valued)
        t_lnc = wpool.tile([128, 1], F32, tag="lnc", name="lnc")
        nc.vector.memset(t_lnc, -LN_SQRT_NB)
        if use_bv:
            t_bv = wpool.tile([1, 256], BF16, tag="bv", name="bv")
            nc.sync.dma_start(out=t_bv, in_=bv_d[:, :])
            ones_row0 = t_consts[0:1, 4:132]  # [1,128] ones at partition 0
        if use_mask:
            t_valid = wpool.tile([128, TOK_CH], F32, tag="valid", name="valid")
            nc.sync.dma_start(out=t_valid, in_=valid_d[:, :])

        # qk[m]: feature-major q/k, bf16; m=0,1 -> q heads (0,1),(2,3);
        # m=2,3 -> k heads (0,1),(2,3)
        t_qk = [qkpool.tile([128, N], BF16, tag=f"qk{m}", name=f"qk{m}") for m in range(4)]
        # kv (transposed) + ksum col 64, bf16, per head -- filled in phase A
        t_kvTs = [kvsp.tile([128, 2, 65], BF16, tag=f"kvTs{h}", name=f"kvTs{h}") for h in range(4)]


        # ---- phase A: S1a projections + k-side features + kv --------
        with tc.tile_pool(name="xt", bufs=1) as xtp, \
             tc.tile_pool(name="worka", bufs=2) as wka, \
             tc.tile_pool(name="psA", bufs=2, space="PSUM") as psA:

            # xt in column-quarters, dispatched from the (idle) ACT queue:
            # the first S1a-k blocks only need the first 1024 tokens, and
            # the SP queue stays free for the weight DMAs
            t_xt = [[xtp.tile([128, N // 4], BF16, tag=f"xt{k}_{hf}", name=f"xt{k}_{hf}")
                     for hf in range(4)] for k in range(4)]
            for hf in range(4):
                for k in range(4):
                    nc.scalar.dma_start(
                        out=t_xt[k][hf],
                        in_=xT[128 * k:128 * (k + 1),
                               1024 * hf:1024 * (hf + 1)])

            def xtap(k, sl):
                hf, lo = divmod(sl.start, 1024)
                return t_xt[k][hf][:, lo:lo + (sl.stop - sl.start)]

            # kv accumulators, two heads packed per bank:
            # [128 m(j-half), 2 head-in-pair, 2 j, 64 d + ksum]
            t_kv2 = [psA.tile([128, 2, 2, 65], F32, tag=f"kv{g}", name=f"kv{g}", bufs=1)
                     for g in range(2)]

            def s1a_block(m, t8, on_dve=False):
                sl = slice(512 * t8, 512 * (t8 + 1))
                ps = psA.tile([128, 512], F32, tag="s1a", name=f"s1a_{m}_{t8}")
                for k in range(4):
                    nc.tensor.matmul(
                        ps,
                        lhsT=t_wqk[k][:, 128 * m:128 * (m + 1)],
                        rhs=xtap(k, sl),
                        start=(k == 0),
                        stop=(k == 3),
                    )
                if on_dve and not use_bqk:
                    nc.vector.tensor_copy(out=t_qk[m][:, sl], in_=ps)
                else:
                    nc.scalar.activation(
                        out=t_qk[m][:, sl], in_=ps,
                        func=AF.Identity,
                        bias=t_bqk[:, m:m + 1], scale=1.0,
                    )

            # k-feature blocks first, quarter-major so compute starts as
            # soon as the first xt quarter lands; q blocks are interleaved
            # into the chunk loop below as PE filler
            for q in range(4):
                for m in (2, 3):
                    for t8 in (2 * q, 2 * q + 1):
                        s1a_block(m, t8)

            s1aq = [(m, t8) for m in (0, 1) for t8 in range(TOK_B)]
            for t in range(TOK_CH):
                cl = slice(128 * t, 128 * (t + 1))
                # v chunk token-major (cols 0:256); sum k^2 in 256:260
                pv = psA.tile([128, 260], F32, tag="pv", name=f"pv{t}")
                for k in range(4):
                    nc.tensor.matmul(
                        pv[:, 0:256],
                        lhsT=xtap(k, cl), rhs=t_wv[k],
                        start=(k == 0), stop=(k == 3) and not use_bv,
                    )
                if use_bv:
                    nc.tensor.matmul(
                        pv[:, 0:256],
                        lhsT=ones_row0, rhs=t_bv,
                        start=False, stop=True,
                    )
                # proj_k token-major via blockdiag omega: [128 tok, 4h*128m]
                pk = psA.tile([128, 512], F32, tag="pk", name=f"pk{t}")
                for p in range(2):
                    nc.tensor.matmul(
                        pk[:, 256 * p:256 * (p + 1)],
                        lhsT=t_qk[2 + p][:, cl],
                        rhs=t_womk[:, 256 * p:256 * (p + 1)],
                        start=True, stop=True,
                    )
                # sum k^2 per token/head via ones-matmul on squared kT chunk
                # (squares on the otherwise-idle GPSIMD engine)
                ksq = wka.tile([128, 2, 128], BF16, tag="ksq", name=f"ksq{t}")
                for p in range(2):
                    nc.gpsimd.tensor_tensor(
                        out=ksq[:, p, :],
                        in0=t_qk[2 + p][:, cl], in1=t_qk[2 + p][:, cl],
                        op=ALU.mult,
                    )
                for p in range(2):
                    nc.tensor.matmul(
                        pv[:, 256 + 2 * p:258 + 2 * p],
                        lhsT=ksq[:, p, :], rhs=ones_blk,
                        start=True, stop=True, skip_group_check=True,
                    )
                # shift_k = absmax_m proj (per token, head)
                srd = wka.tile([128, 4], F32, tag="srd", name=f"srd{t}")
                nc.vector.tensor_reduce(
                    out=srd,
                    in_=pk.rearrange("p (h m) -> p h m", h=4),
                    axis=AX.X, op=ALU.max,
                    apply_absolute_value=True,
                )
                # bk0 = -c*srd - SSQ_C*sumk2 (the ssq_blk "ones" carry
                # SSQ_C); ebk = exp(bk0 - ln sqrt(NB)) via the ACT bias
                bk = wka.tile([128, 4], F32, tag="bk", name=f"bk{t}")
                nc.vector.scalar_tensor_tensor(
                    out=bk, in0=srd, scalar=-INV_DKRT, in1=pv[:, 256:260],
                    op0=ALU.mult, op1=ALU.subtract,
                )
                ebk = wka.tile([128, 4, 1], BF16, tag="ebk", name=f"ebk{t}")
                nc.scalar.activation(
                    out=ebk, in_=bk, func=AF.Exp,
                    bias=t_lnc, scale=1.0,
                )
                if use_mask:
                    nc.vector.tensor_scalar_mul(
                        ebk.rearrange("p h o -> p (h o)"),
                        ebk.rearrange("p h o -> p (h o)"),
                        t_valid[:, t:t + 1])
                # scale v rows by ebk per head: one DVE broadcast-multiply
                # straight from PSUM
                va = wka.tile([128, 4, 65], BF16, tag="va", name=f"va{t}")
                nc.vector.tensor_tensor(
                    out=va[:, :, 0:64],
                    in0=pv[:, 0:256].rearrange("p (h d) -> p h d", h=4),
                    in1=ebk.to_broadcast((128, 4, 64)),
                    op=ALU.mult,
                )
                nc.gpsimd.tensor_copy(
                    out=va[:, :, 64:65].rearrange("p h o -> p (h o)"),
                    in_=ebk.rearrange("p h o -> p (h o)"))
                # phi_k = exp(+-c*proj), unbiased, bf16
                kph = wka.tile([128, 4, 256], BF16, tag="kph", name=f"kph{t}")
                nc.scalar.activation(
                    out=kph[:, :, 0:128],
                    in_=pk.rearrange("p (h m) -> p h m", h=4),
                    func=AF.Exp, bias=0.0, scale=INV_DKRT,
                )
                nc.scalar.activation(
                    out=kph[:, :, 128:256],
                    in_=pk.rearrange("p (h m) -> p h m", h=4),
                    func=AF.Exp, bias=0.0, scale=-INV_DKRT,
                )
                # kv accumulation, directly transposed:
                # kvT[m, (d|ksum)] += phi[tok, m].T @ va[tok, (d|ksum)]
                # start only on the bank's first group: its start marks the
                # whole 2KB bank pending-zero, initializing all 4 groups --
                # a second start would re-mark (and discard) earlier writes
                for h in range(4):
                    for j in range(2):
                        nc.tensor.matmul(
                            t_kv2[h // 2][:, h % 2, j, :],
                            lhsT=kph[:, h, 128 * j:128 * (j + 1)],
                            rhs=va[:, h, :],
                            start=(t == 0 and h % 2 == 0 and j == 0),
                            stop=(t == TOK_CH - 1),
                            skip_group_check=True,
                        )
                if debug and t == 0:
                    dpk = wka.tile([128, 512], F32, tag="dpk", name="dpk")
                    nc.vector.tensor_copy(out=dpk, in_=pk)
                    nc.sync.dma_start(out=dbg_ch[0], in_=dpk)
                    dpv = wka.tile([128, 512], F32, tag="dpk", name="dpv")
                    nc.vector.tensor_copy(out=dpv[:, 0:260], in_=pv)
                    nc.sync.dma_start(out=dbg_ch[1][:, 0:260], in_=dpv[:, 0:260])
                    dsm = wka.tile([128, 512], F32, tag="dpk", name="dsm")
                    nc.vector.tensor_copy(out=dsm[:, 0:4], in_=srd)
                    nc.vector.tensor_copy(out=dsm[:, 8:12], in_=bk)
                    nc.vector.tensor_copy(
                        out=dsm[:, 12:16],
                        in_=ebk.rearrange("p h o -> p (h o)"))
                    nc.sync.dma_start(out=dbg_ch[2][:, 0:16], in_=dsm[:, 0:16])
                    dkb = wka.tile([128, 1024], BF16, tag="dkb", name="dkb")
                    nc.vector.tensor_copy(
                        out=dkb[:, 256:516],
                        in_=va.rearrange("p a b -> p (a b)"))
                    nc.sync.dma_start(out=dbg_b[6][:, 256:516], in_=dkb[:, 256:516])
                    dk2 = wka.tile([128, 1024], BF16, tag="dkb", name="dk2")
                    nc.vector.tensor_copy(
                        out=dk2, in_=kph.rearrange("p a b -> p (a b)"))
                    nc.sync.dma_start(out=dbg_b[7], in_=dk2)
                # PE filler: S1a q-feature blocks, 1 per odd chunk
                if t % 2 == 1 and s1aq:
                    s1a_block(*s1aq.pop(0), on_dve=True)

            while s1aq:
                s1a_block(*s1aq.pop(0), on_dve=True)

            for h in range(4):
                nc.vector.tensor_copy(out=t_kvTs[h], in_=t_kv2[h // 2][:, h % 2, :, :])
            if debug:
                for m in range(4):
                    nc.sync.dma_start(out=dbg_qk[m], in_=t_qk[m])
                for h in range(4):
                    nc.sync.dma_start(
                        out=dbg_kvs[h],
                        in_=t_kvTs[h].rearrange("p a b -> p (a b)"))

        # ---- phase B: q-side features, num/den, output --------------
        with tc.tile_pool(name="workb", bufs=2) as wkb, \
             tc.tile_pool(name="drb", bufs=2, space="DRAM") as drb, \
             tc.tile_pool(name="psB", bufs=2, space="PSUM") as psB:
            # den4/rcp4 ring slots: unused partition rows must hold a
            # finite value (the Ln/Exp pass covers all 128 rows)
            den4s = [wkb.tile([128, 512], F32, tag="den4", name=f"den4_{i}")
                     for i in range(2)]
            for i in range(2):
                nc.vector.memset(den4s[i], 1.0)
            for t8 in range(TOK_B):
                sl = slice(512 * t8, 512 * (t8 + 1))
                pns = []
                den4 = den4s[t8 % 2]
                nsrs = [wkb.tile([128, 512], BF16, tag="nsr", name=f"nsr{t8}_{d}", bufs=4)
                        for d in range(2)]
                for h in range(4):
                    pq = psB.tile([128, 512], F32, tag="pq", name=f"pq{t8}_{h}", bufs=2)
                    nc.tensor.matmul(
                        pq,
                        lhsT=t_womq[:, 128 * h:128 * (h + 1)],
                        rhs=t_qk[h // 2][:, sl],
                        start=True, stop=True,
                    )
                    qp = wkb.tile([128, 2, 512], BF16, tag="qp", name=f"qp{t8}_{h}")
                    nc.scalar.activation(
                        out=qp[:, 0, :], in_=pq,
                        func=AF.Exp, bias=0.0, scale=INV_DKRT,
                    )
                    nc.scalar.activation(
                        out=qp[:, 1, :], in_=pq,
                        func=AF.Exp, bias=0.0, scale=-INV_DKRT,
                    )
                    pn = psB.tile([65, 512], F32, tag="pn", name=f"pn{t8}_{h}", bufs=2)
                    for j in range(2):
                        nc.tensor.matmul(
                            pn,
                            lhsT=t_kvTs[h][:, j, :], rhs=qp[:, j, :],
                            start=(j == 0), stop=(j == 1),
                        )
                    # gather den rows at 32-aligned partitions (1/den is
                    # exp(-ln(den)) on ACT: ln+exp share one table with
                    # exp/identity, and DVE reciprocal costs 3.3us); copy
                    # num out of PSUM immediately to free the pn ring
                    nc.vector.tensor_copy(
                        out=den4[32 * h:32 * h + 1, :], in_=pn[64:65, :])
                    nc.vector.tensor_copy(
                        out=nsrs[h // 2][64 * (h % 2):64 * (h % 2) + 64, :],
                        in_=pn[0:64, :])
                    if debug and t8 == 0:
                        nc.sync.dma_start(
                            out=dbg_b[h],
                            in_=qp.rearrange("p a b -> p (a b)"))
                lnd = wkb.tile([128, 512], F32, tag="lnd", name=f"lnd{t8}")
                nc.scalar.activation(
                    out=lnd, in_=den4, func=AF.Ln, bias=0.0, scale=1.0,
                )
                rcp4 = wkb.tile([128, 512], BF16, tag="rcp4", name=f"rcp4{t8}")
                nc.scalar.activation(
                    out=rcp4, in_=lnd, func=AF.Exp, bias=0.0, scale=-1.0,
                )
                ns = []
                for d in range(2):
                    # broadcast 1/den across partitions with K=1 matmuls
                    # (ones row at partition 32h selects the rcp4 row);
                    # no DRAM bounce, no DMA latency on the critical path
                    bc = psB.tile([128, 512], F32, tag="bc", name=f"bc{t8}_{d}", bufs=2)
                    for i in range(2):
                        h = 2 * d + i
                        nc.tensor.matmul(
                            bc[64 * i:64 * (i + 1), :],
                            lhsT=t_consts[32 * h:32 * h + 1, 4:68],
                            rhs=rcp4[32 * h:32 * h + 1, :],
                            start=True, stop=True, skip_group_check=True,
                            tile_position=(32 * h, 64 * i),
                        )
                    nst = wkb.tile([128, 512], BF16, tag="ns", name=f"ns{t8}_{d}", bufs=4)
                    nc.vector.tensor_tensor(
                        out=nst, in0=nsrs[d], in1=bc, op=ALU.mult,
                    )
                    ns.append(nst)
                for m4 in range(4):
                    py = psB.tile([128, 512], F32, tag="py", name=f"py{t8}_{m4}")
                    for dd in range(2):
                        nc.tensor.matmul(
                            py,
                            lhsT=t_wy[dd][:, 128 * m4:128 * (m4 + 1)],
                            rhs=ns[dd],
                            start=(dd == 0), stop=(dd == 1),
                        )
                    ysb = wkb.tile([128, 512], BF16, tag="ysb", name=f"ysb{t8}_{m4}", bufs=4)
                    if m4 % 2 == 0:
                        nc.vector.tensor_copy(out=ysb, in_=py)
                    else:
                        nc.scalar.copy(out=ysb, in_=py)
                    nc.sync.dma_start(
                        out=yT[128 * m4:128 * (m4 + 1), sl], in_=ysb,
                    )
                if debug and t8 == 0:
                    nc.sync.dma_start(out=dbg_b[4][:, 0:512], in_=ns[0])
                    nc.sync.dma_start(out=dbg_b[5][:, 0:512], in_=ns[1])

    if split:
        _split_waits(nc)
    return nc


_PROGRAM_CACHE = {}


def _get_program(use_bv, use_mask, use_bqk):
    key = (use_bv, use_mask, use_bqk)
    if key not in _PROGRAM_CACHE:
        _PROGRAM_CACHE[key] = build_program(*key)
    return _PROGRAM_CACHE[key]


def _bf(a):
    return np.ascontiguousarray(a).astype(ml_dtypes.bfloat16)


def make_in_maps(x, key_padding_mask, Wqkv, bqkv, Wout, bout, omega):
    """Shard + lay out the full inputs into 8 per-core input maps."""
    Wq, Wk, Wv = Wqkv[0:D], Wqkv[D:2 * D], Wqkv[2 * D:3 * D]
    bq, bk_, bv = bqkv[0:D], bqkv[D:2 * D], bqkv[2 * D:3 * D]
    mask = key_padding_mask

    use_bv = bool(np.any(bv != 0))
    use_mask = bool(np.any(mask))
    use_bqk = bool(np.any(bq != 0) or np.any(bk_ != 0))

    consts = np.zeros((128, 132), np.float32)
    consts[0:64, 0] = SSQ_C
    consts[64:128, 1] = SSQ_C
    for r in (0, 32, 64, 96):
        consts[r, 4:132] = 1.0

    in_maps = []
    for c in range(8):
        b, hg = c // 2, c % 2
        dsl = slice(256 * hg, 256 * (hg + 1))
        heads = [4 * hg + i for i in range(4)]
        wqk_c = np.concatenate([Wq.T[:, dsl], Wk.T[:, dsl]], axis=1)
        womq_c = np.zeros((128, 512), np.float32)
        womk_c = np.zeros((128, 512), np.float32)
        for i, g in enumerate(heads):
            off = 64 * (i % 2)
            womq_c[off:off + 64, 128 * i:128 * (i + 1)] = omega[g].T
        for p in range(2):
            womk_c[0:64, 256 * p:256 * p + 128] = omega[heads[2 * p]].T
            womk_c[64:128, 256 * p + 128:256 * p + 256] = omega[heads[2 * p + 1]].T
        bqk_vec = np.concatenate([bq[dsl], bk_[dsl]])
        im = {
            "xT": _bf(x[b].T),
            "wqk": _bf(wqk_c),
            "wv": _bf(Wv.T[:, dsl]),
            "womq": _bf(womq_c),
            "womk": _bf(womk_c),
            "wy": _bf(Wout[:, dsl].T),
            "bqk": np.ascontiguousarray(bqk_vec.reshape(4, 128).T),
            "consts": _bf(consts),
        }
        if use_bv:
            im["bv"] = _bf(bv[None, :])
        if use_mask:
            im["valid"] = np.ascontiguousarray(
                (~mask[b]).astype(np.float32).reshape(TOK_CH, 128).T
            )
        in_maps.append(im)
    return in_maps, (use_bv, use_mask, use_bqk)


def gather_output(per_core_yT, bout):
    """Sum head-group partials, transpose back to (B, N, D), add bout."""
    y = np.empty((B, N, D), np.float32)
    for b in range(B):
        acc = (per_core_yT[2 * b].astype(np.float32)
               + per_core_yT[2 * b + 1].astype(np.float32))
        y[b] = acc.T
    if np.any(bout != 0):
        y += bout[None, None, :]
    return y


def kernel(x, key_padding_mask, Wqkv, bqkv, Wout, bout, omega):
    from concourse.bass_utils import run_bass_kernel_spmd

    x = np.asarray(x, np.float32)
    mask = np.asarray(key_padding_mask)
    Wqkv = np.asarray(Wqkv, np.float32)
    bqkv = np.asarray(bqkv, np.float32)
    Wout = np.asarray(Wout, np.float32)
    bout = np.asarray(bout, np.float32)
    omega = np.asarray(omega, np.float32)

    in_maps, flags = make_in_maps(x, mask, Wqkv, bqkv, Wout, bout, omega)
    nc = _get_program(*flags)
    res = run_bass_kernel_spmd(nc, in_maps, list(range(8)))
    return gather_output([r["yT"] for r in res.results], bout)
